# revision 19
# baseline (speedup 1.0000x reference)
"""Trainium2 Bass kernel for APGLinear (hypernet-generated per-sample Linear).

Reference computation (B=4096, IN=256, OUT=256, COND=128, HID=512):
    hyp      = relu(condition_z @ W1 + b1)            # [B, HID]
    weight_S = (hyp @ W2 + b2).reshape(B, IN, OUT)    # 1 GB intermediate
    out      = einsum("bi,bio->bo", input_h, weight_S) + bias

Strategy:
  * Shard OUT across the 8 cores (32 columns each). Each core needs only a
    16 MB slice of W2, all of z / input_h, and computes the full hypernet
    activations locally (cheap). No collectives; host concatenates outputs.
  * The big matmul hyp @ W2_slice runs on the TensorEngine in bf16 with
    hyp^T tiles stationary; per-sample weights exist only in PSUM chunks of
    [128 samples, 2 out-cols x 256 in] (o-major layout).
  * The per-sample contraction sum_i input_h[b,i] * wS[b,i,o] is a fused
    multiply+reduce on the Vector engine (tensor_tensor_reduce), one
    instruction per (batch-tile, out-col), accumulator seeded with the
    (input_h @ b2 + bias) term computed by a tiny augmented matmul.
"""

import os
import sys

import numpy as np

for _p in ("/opt/trn_rl_repo",):
    if os.path.isdir(_p) and _p not in sys.path:
        sys.path.append(_p)

import ml_dtypes  # noqa: E402

import concourse.bass as bass  # noqa: E402,F401
import concourse.tile as tile  # noqa: E402
from concourse import bacc, mybir  # noqa: E402
from concourse.bass_utils import run_bass_kernel_spmd  # noqa: E402

B = 4096
COND = 128
IN = 256
OUT = 256
HID = 512
NCORES = 8
OS = OUT // NCORES  # 32 out-cols per core
KH = HID // 128  # 4 contraction tiles for the big matmul
CH = 512  # psum chunk columns (= one PSUM bank of fp32)
OPC = CH // IN  # out-cols per chunk = 2

MM_DT = mybir.dt.bfloat16
NP_MM_DT = ml_dtypes.bfloat16
F32 = mybir.dt.float32


def build(b=B, variant="v2mr"):
    """Build + compile the SPMD single-core program (same on all 8 cores).

    variant: "full" | "scalar0" (ttr initial = 0.0 float) | "nottr" (skip the
    vector-engine reduce entirely; output is garbage — crash bisection only).
    """
    nt = b // 128  # batch tiles
    hbc = 512  # hyp-phase batch chunk (moving-operand cols)
    nbc = max(1, b // hbc)
    nch = (IN * OS) // CH  # 16 chunks per batch tile

    nc = bacc.Bacc("TRN2", target_bir_lowering=False, debug=False)

    zt_d = nc.dram_tensor("zt", [COND, b], MM_DT, kind="ExternalInput").ap()
    w1_d = nc.dram_tensor("w1", [COND, HID], MM_DT, kind="ExternalInput").ap()
    b1_d = nc.dram_tensor("b1t", [128, KH], F32, kind="ExternalInput").ap()
    # input_h duplicated along cols ([inh, inh]) in bf16, for the fused
    # multiply-reduce against o-major wS chunks
    inh_d = nc.dram_tensor("inhb", [b, CH], MM_DT, kind="ExternalInput").ap()
    ita_d = nc.dram_tensor("inhta", [IN + 128, b], MM_DT, kind="ExternalInput").ap()
    b2a_d = nc.dram_tensor("b2a", [IN + 128, OS], MM_DT, kind="ExternalInput").ap()
    w2_d = nc.dram_tensor("w2s", [HID, IN * OS], MM_DT, kind="ExternalInput").ap()
    out_d = nc.dram_tensor("out", [b, OS], F32, kind="ExternalOutput").ap()

    relu = mybir.ActivationFunctionType.Relu
    mult = mybir.AluOpType.mult
    add = mybir.AluOpType.add

    with tile.TileContext(nc) as tc:
        with tc.tile_pool(name="persist", bufs=1) as pp:
            zt = pp.tile([COND, b], MM_DT)
            nc.sync.dma_start(zt, zt_d)
            w1 = pp.tile([COND, HID], MM_DT)
            nc.sync.dma_start(w1, w1_d)
            b1t = pp.tile([128, KH], F32)
            nc.sync.dma_start(b1t, b1_d)
            ita0 = pp.tile([128, b], MM_DT)
            nc.sync.dma_start(ita0, ita_d[0:128, :])
            ita1 = pp.tile([128, b], MM_DT)
            nc.sync.dma_start(ita1, ita_d[128:256, :])
            ita2 = pp.tile([128, b], MM_DT)
            nc.sync.dma_start(ita2, ita_d[256:384, :])
            b2a0 = pp.tile([128, OS], MM_DT)
            nc.sync.dma_start(b2a0, b2a_d[0:128, :])
            b2a1 = pp.tile([128, OS], MM_DT)
            nc.sync.dma_start(b2a1, b2a_d[128:256, :])
            b2a2 = pp.tile([128, OS], MM_DT)
            nc.sync.dma_start(b2a2, b2a_d[256:384, :])

            inh_all = pp.tile([128, nt * CH], MM_DT)
            for t in range(nt):
                nc.sync.dma_start(
                    inh_all[:, t * CH : (t + 1) * CH], inh_d[t * 128 : (t + 1) * 128, :]
                )

            hyps = [pp.tile([128, b], MM_DT, name=f"hypt{k}") for k in range(KH)]
            oacc = pp.tile([128, nt * OS], F32)
            cterm = pp.tile([128, nt * OS], F32)

            # ---- hypernet layer 1: hyp^T = relu(W1^T @ z^T + b1) ----
            with tc.tile_pool(name="hpsum", bufs=4, space="PSUM") as hpp:
                for k in range(KH):
                    for c2 in range(nbc):
                        hp = hpp.tile([128, min(hbc, b)], F32, tag="hp")
                        bs = slice(c2 * hbc, c2 * hbc + min(hbc, b))
                        nc.tensor.matmul(
                            hp,
                            w1[:, k * 128 : (k + 1) * 128],
                            zt[:, bs],
                            start=True,
                            stop=True,
                        )
                        nc.scalar.activation(
                            hyps[k][:, bs], hp, relu, bias=b1t[:, k : k + 1], scale=1.0
                        )
                # ---- c-term: cterm = input_h @ b2_slice + bias_slice ----
                for t in range(nt):
                    cp = hpp.tile([128, OS], F32, tag="cp")
                    bsl = slice(t * 128, (t + 1) * 128)
                    nc.tensor.matmul(cp, ita0[:, bsl], b2a0, start=True, stop=False)
                    nc.tensor.matmul(cp, ita1[:, bsl], b2a1, start=False, stop=False)
                    nc.tensor.matmul(cp, ita2[:, bsl], b2a2, start=False, stop=True)
                    nc.vector.tensor_copy(cterm[:, t * OS : (t + 1) * OS], cp)

            # ---- main: wS chunks in PSUM -> bf16 SBUF (ScalarE) -> fused
            # multiply(+reduce) on the Vector engine ----
            scratch = pp.tile([128, CH], MM_DT)
            copyf = mybir.ActivationFunctionType.Copy
            with (
                tc.tile_pool(name="w2p", bufs=3) as w2p,
                tc.tile_pool(name="wsp", bufs=4) as wsp,
                tc.tile_pool(name="mp", bufs=6, space="PSUM") as mp,
            ):
                oacc3 = oacc.rearrange("p (t o) -> p t o", o=OS)
                for c in range(nch):
                    w2c = w2p.tile([128, KH * CH], MM_DT)
                    for k in range(KH):
                        nc.sync.dma_start(
                            w2c[:, k * CH : (k + 1) * CH],
                            w2_d[k * 128 : (k + 1) * 128, c * CH : (c + 1) * CH],
                        )
                    if variant == "v3":
                        # epilogue fused across batch-tile pairs: one mult +
                        # one segmented reduce per 2 chunks of work
                        for tp in range(nt // 2):
                            wsb2 = wsp.tile([128, 2 * CH], MM_DT, tag="wsb2")
                            for tsub in range(2):
                                t = 2 * tp + tsub
                                ps = mp.tile([128, CH], F32)
                                for k in range(KH):
                                    nc.tensor.matmul(
                                        ps,
                                        hyps[k][:, t * 128 : (t + 1) * 128],
                                        w2c[:, k * CH : (k + 1) * CH],
                                        start=(k == 0),
                                        stop=(k == KH - 1),
                                    )
                                nc.scalar.activation(
                                    wsb2[:, tsub * CH : (tsub + 1) * CH], ps, copyf
                                )
                            prod2 = wsp.tile([128, 2 * CH], MM_DT, tag="prod2")
                            nc.vector.tensor_mul(
                                prod2,
                                wsb2,
                                inh_all[:, 2 * tp * CH : (2 * tp + 2) * CH],
                            )
                            nc.vector.tensor_reduce(
                                oacc3[:, 2 * tp : 2 * tp + 2, OPC * c : OPC * (c + 1)],
                                prod2.rearrange("p (t o i) -> p t o i", o=OPC, i=IN),
                                axis=mybir.AxisListType.X,
                                op=add,
                            )
                        continue
                    for t in range(nt):
                        ps = mp.tile([128, CH], F32)
                        for k in range(KH):
                            nc.tensor.matmul(
                                ps,
                                hyps[k][:, t * 128 : (t + 1) * 128],
                                w2c[:, k * CH : (k + 1) * CH],
                                start=(k == 0),
                                stop=(k == KH - 1),
                            )
                        wsb = wsp.tile([128, CH], MM_DT)
                        nc.scalar.activation(wsb, ps, copyf)
                        use_ttr = variant == "v2ttr" or (
                            variant == "v2both" and t % 2 == 0
                        )
                        if use_ttr:
                            for orel in range(OPC):
                                o = OPC * c + orel
                                nc.vector.tensor_tensor_reduce(
                                    scratch[:, orel * IN : (orel + 1) * IN],
                                    wsb[:, orel * IN : (orel + 1) * IN],
                                    inh_all[:, t * CH + orel * IN : t * CH + (orel + 1) * IN],
                                    scale=1.0,
                                    scalar=0.0,
                                    op0=mult,
                                    op1=add,
                                    accum_out=oacc[:, t * OS + o : t * OS + o + 1],
                                )
                        else:
                            prod = wsp.tile([128, CH], MM_DT, tag="prod")
                            nc.vector.tensor_mul(
                                prod, wsb, inh_all[:, t * CH : (t + 1) * CH]
                            )
                            nc.vector.tensor_reduce(
                                oacc[:, t * OS + OPC * c : t * OS + OPC * (c + 1)],
                                prod.rearrange("p (o i) -> p o i", i=IN),
                                axis=mybir.AxisListType.X,
                                op=add,
                            )
            # final: add the (input_h @ b2 + bias) term
            nc.vector.tensor_add(oacc, oacc, cterm)

            for t in range(nt):
                nc.sync.dma_start(
                    out_d[t * 128 : (t + 1) * 128, :], oacc[:, t * OS : (t + 1) * OS]
                )

    nc.compile()
    return nc


def make_in_maps(inputs, b=B):
    """Host-side input prep: layout shuffles + dtype casts, per-core shards."""
    inh = np.asarray(inputs["input_h"], dtype=np.float32)
    z = np.asarray(inputs["condition_z"], dtype=np.float32)
    W1 = np.asarray(inputs["W1"], dtype=np.float32)
    b1 = np.asarray(inputs["b1"], dtype=np.float32)
    W2 = np.asarray(inputs["W2"], dtype=np.float32)
    b2 = np.asarray(inputs["b2"], dtype=np.float32)
    bias = np.asarray(inputs["bias"], dtype=np.float32).reshape(1, OUT)

    bf = NP_MM_DT
    zt = np.ascontiguousarray(z.T).astype(bf)  # [COND, b]
    w1h = W1.astype(bf)  # [COND, HID]
    b1t = np.ascontiguousarray(b1.reshape(KH, 128).T).astype(np.float32)  # [128, KH]
    inhb = np.concatenate([inh, inh], axis=1).astype(bf)  # [b, 2*IN]
    # pad the augmented rows to a full 128-partition k-tile (row 256 = ones,
    # rows 257..383 = zeros) so every SBUF tile/matmul uses 128 partitions
    ita = np.concatenate(
        [inh.T, np.ones((1, b), np.float32), np.zeros((127, b), np.float32)], axis=0
    ).astype(bf)
    W2r = W2.reshape(HID, IN, OUT)
    b2r = b2.reshape(IN, OUT)

    in_maps = []
    for c in range(NCORES):
        osl = slice(c * OS, (c + 1) * OS)
        # o-major columns: col = o_rel * IN + i
        w2s = (
            np.ascontiguousarray(W2r[:, :, osl].transpose(0, 2, 1))
            .reshape(HID, OS * IN)
            .astype(bf)
        )
        b2a = np.concatenate(
            [b2r[:, osl], bias[:, osl], np.zeros((127, OS), np.float32)], axis=0
        ).astype(bf)
        in_maps.append(
            {
                "zt": zt,
                "w1": w1h,
                "b1t": b1t,
                "inhb": inhb,
                "inhta": ita,
                "b2a": b2a,
                "w2s": w2s,
            }
        )
    return in_maps


_NC_CACHE = {}


def get_nc(b=B):
    if b not in _NC_CACHE:
        _NC_CACHE[b] = build(b)
    return _NC_CACHE[b]


def kernel(**inputs) -> np.ndarray:
    nc = get_nc(B)
    in_maps = make_in_maps(inputs, B)
    res = run_bass_kernel_spmd(nc, in_maps, core_ids=list(range(NCORES)))
    out = np.concatenate(
        [res.results[c]["out"] for c in range(NCORES)], axis=1
    )
    return np.ascontiguousarray(out.astype(np.float32))


# revision 29
# speedup vs baseline: 1.2905x; 1.2905x over previous
"""Trainium2 Bass kernel for APGLinear (hypernet-generated per-sample Linear).

Reference computation (B=4096, IN=256, OUT=256, COND=128, HID=512):
    hyp      = relu(condition_z @ W1 + b1)            # [B, HID]
    weight_S = (hyp @ W2 + b2).reshape(B, IN, OUT)    # 1 GB intermediate
    out      = einsum("bi,bio->bo", input_h, weight_S) + bias

Strategy:
  * Shard OUT across the 8 cores (32 columns each). Each core needs only a
    16 MB slice of W2, all of z / input_h, and computes the full hypernet
    activations locally (cheap). No collectives; host concatenates outputs.
  * The big matmul hyp @ W2_slice runs on the TensorEngine in bf16 with
    hyp^T tiles stationary; per-sample weights exist only in PSUM chunks of
    [128 samples, 2 out-cols x 256 in] (o-major layout).
  * Each PSUM chunk is downcast to bf16 SBUF by the Scalar engine, then the
    per-sample contraction sum_i input_h[b,i] * wS[b,i,o] runs on the Vector
    engine as tensor_mul + segmented tensor_reduce (axis=X over a 3D view).
    (tensor_tensor_reduce would fuse these but faults on this HW path.)
    The (input_h @ b2 + bias) term comes from a tiny augmented matmul and is
    added at the end.
"""

import os
import sys

import numpy as np

for _p in ("/opt/trn_rl_repo",):
    if os.path.isdir(_p) and _p not in sys.path:
        sys.path.append(_p)

import ml_dtypes  # noqa: E402

import concourse.bass as bass  # noqa: E402,F401
import concourse.tile as tile  # noqa: E402
from concourse import bacc, mybir  # noqa: E402
from concourse.bass_utils import run_bass_kernel_spmd  # noqa: E402

B = 4096
COND = 128
IN = 256
OUT = 256
HID = 512
NCORES = 8
OS = OUT // NCORES  # 32 out-cols per core
KH = HID // 128  # 4 contraction tiles for the big matmul
CH = 512  # psum chunk columns (= one PSUM bank of fp32)
OPC = CH // IN  # out-cols per chunk = 2

MM_DT = mybir.dt.bfloat16
NP_MM_DT = ml_dtypes.bfloat16
F32 = mybir.dt.float32


def build(b=B, variant="v7"):
    """Build + compile the SPMD single-core program (same on all 8 cores).

    variant: "v2mr" (default, HW-validated: mul + segmented reduce) |
    "v2ttr"/"v2both" (tensor_tensor_reduce — crashes real HW, kept for
    reference) | "v3" (pair-fused epilogue — measured slower) | "hyponly".
    """
    nt = b // 128  # batch tiles
    hbc = 512  # hyp-phase batch chunk (moving-operand cols)
    nbc = max(1, b // hbc)
    nch = (IN * OS) // CH  # 16 chunks per batch tile

    nc = bacc.Bacc("TRN2", target_bir_lowering=False, debug=False)

    zt_d = nc.dram_tensor("zt", [COND, b], MM_DT, kind="ExternalInput").ap()
    w1_d = nc.dram_tensor("w1", [COND, HID], MM_DT, kind="ExternalInput").ap()
    b1_d = nc.dram_tensor("b1t", [128, KH], F32, kind="ExternalInput").ap()
    # input_h duplicated along cols ([inh, inh]) in bf16, for the fused
    # multiply-reduce against o-major wS chunks
    inh_d = nc.dram_tensor("inhb", [b, CH], MM_DT, kind="ExternalInput").ap()
    ita_d = nc.dram_tensor("inhta", [IN + 128, b], MM_DT, kind="ExternalInput").ap()
    b2a_d = nc.dram_tensor("b2a", [IN + 128, OS], MM_DT, kind="ExternalInput").ap()
    w2_d = nc.dram_tensor("w2s", [HID, IN * OS], MM_DT, kind="ExternalInput").ap()
    out_d = nc.dram_tensor("out", [b, OS], F32, kind="ExternalOutput").ap()

    relu = mybir.ActivationFunctionType.Relu
    mult = mybir.AluOpType.mult
    add = mybir.AluOpType.add

    with tile.TileContext(nc) as tc:
        with tc.tile_pool(name="persist", bufs=1) as pp:
            zt = pp.tile([COND, b], MM_DT)
            nc.sync.dma_start(zt, zt_d)
            w1 = pp.tile([COND, HID], MM_DT)
            nc.sync.dma_start(w1, w1_d)
            b1t = pp.tile([128, KH], F32)
            nc.sync.dma_start(b1t, b1_d)
            ita0 = pp.tile([128, b], MM_DT)
            nc.sync.dma_start(ita0, ita_d[0:128, :])
            ita1 = pp.tile([128, b], MM_DT)
            nc.sync.dma_start(ita1, ita_d[128:256, :])
            ita2 = pp.tile([128, b], MM_DT)
            nc.sync.dma_start(ita2, ita_d[256:384, :])
            b2a0 = pp.tile([128, OS], MM_DT)
            nc.sync.dma_start(b2a0, b2a_d[0:128, :])
            b2a1 = pp.tile([128, OS], MM_DT)
            nc.sync.dma_start(b2a1, b2a_d[128:256, :])
            b2a2 = pp.tile([128, OS], MM_DT)
            nc.sync.dma_start(b2a2, b2a_d[256:384, :])

            inh_all = pp.tile([128, nt * CH], MM_DT)
            for t in range(nt):
                nc.sync.dma_start(
                    inh_all[:, t * CH : (t + 1) * CH], inh_d[t * 128 : (t + 1) * 128, :]
                )

            hyps = [pp.tile([128, b], MM_DT, name=f"hypt{k}") for k in range(KH)]
            oacc = pp.tile([128, nt * OS], F32)
            cterm = pp.tile([128, nt * OS], F32)
            if variant == "pe_act":
                nc.vector.memset(oacc, 0.0)

            # ---- hypernet layer 1: hyp^T = relu(W1^T @ z^T + b1) ----
            with tc.tile_pool(name="hpsum", bufs=4, space="PSUM") as hpp:
                for k in range(KH):
                    for c2 in range(nbc):
                        hp = hpp.tile([128, min(hbc, b)], F32, tag="hp")
                        bs = slice(c2 * hbc, c2 * hbc + min(hbc, b))
                        nc.tensor.matmul(
                            hp,
                            w1[:, k * 128 : (k + 1) * 128],
                            zt[:, bs],
                            start=True,
                            stop=True,
                        )
                        nc.scalar.activation(
                            hyps[k][:, bs], hp, relu, bias=b1t[:, k : k + 1], scale=1.0
                        )
                # ---- c-term: cterm = input_h @ b2_slice + bias_slice ----
                for t in range(nt):
                    cp = hpp.tile([128, OS], F32, tag="cp")
                    bsl = slice(t * 128, (t + 1) * 128)
                    nc.tensor.matmul(cp, ita0[:, bsl], b2a0, start=True, stop=False)
                    nc.tensor.matmul(cp, ita1[:, bsl], b2a1, start=False, stop=False)
                    nc.tensor.matmul(cp, ita2[:, bsl], b2a2, start=False, stop=True)
                    nc.vector.tensor_copy(cterm[:, t * OS : (t + 1) * OS], cp)

            # ---- main: wS chunks in PSUM -> bf16 SBUF (ScalarE) -> fused
            # multiply(+reduce) on the Vector engine ----
            scratch = pp.tile([128, CH], MM_DT)
            copyf = mybir.ActivationFunctionType.Copy
            deep = variant in ("v8", "v7")
            with (
                tc.tile_pool(name="w2p", bufs=4 if deep else 3) as w2p,
                tc.tile_pool(name="wsp", bufs=8 if deep else 4) as wsp,
                tc.tile_pool(name="mp", bufs=8 if deep else 6, space="PSUM") as mp,
            ):
                oacc3 = oacc.rearrange("p (t o) -> p t o", o=OS)
                if variant == "v6":
                    # k-grouped: one stationary load covers 4 chunk-matmuls
                    # (4 PSUM banks accumulate in parallel per k step)
                    gsz = 4
                    for g in range(nch // gsz):
                        w2g = w2p.tile([128, KH * gsz * CH], MM_DT, tag="w2g")
                        for k in range(KH):
                            nc.sync.dma_start(
                                w2g[:, k * gsz * CH : (k + 1) * gsz * CH],
                                w2_d[
                                    k * 128 : (k + 1) * 128,
                                    g * gsz * CH : (g + 1) * gsz * CH,
                                ],
                            )
                        for t in range(nt):
                            ps4 = mp.tile([128, gsz * CH], F32, tag="ps4", bufs=2)
                            for k in range(KH):
                                for cc in range(gsz):
                                    nc.tensor.matmul(
                                        ps4[:, cc * CH : (cc + 1) * CH],
                                        hyps[k][:, t * 128 : (t + 1) * 128],
                                        w2g[:, (k * gsz + cc) * CH : (k * gsz + cc + 1) * CH],
                                        start=(k == 0),
                                        stop=(k == KH - 1),
                                        skip_group_check=True,
                                    )
                            for cc in range(gsz):
                                c = g * gsz + cc
                                wsb6 = wsp.tile([128, CH], MM_DT, tag="wsb6")
                                nc.scalar.activation(
                                    wsb6, ps4[:, cc * CH : (cc + 1) * CH], copyf
                                )
                                prod6 = wsp.tile([128, CH], MM_DT, tag="prod6")
                                nc.vector.tensor_mul(
                                    prod6, wsb6, inh_all[:, t * CH : (t + 1) * CH]
                                )
                                nc.vector.tensor_reduce(
                                    oacc3[:, t, OPC * c : OPC * (c + 1)],
                                    prod6.rearrange("p (o i) -> p o i", i=IN),
                                    axis=mybir.AxisListType.X,
                                    op=add,
                                )
                    # the shared cterm add below the loop still runs
                    nch_eff = 0
                else:
                    nch_eff = nch
                for c in range(nch_eff):
                    w2c = w2p.tile([128, KH * CH], MM_DT)
                    for k in range(KH):
                        nc.sync.dma_start(
                            w2c[:, k * CH : (k + 1) * CH],
                            w2_d[k * 128 : (k + 1) * 128, c * CH : (c + 1) * CH],
                        )
                    if variant == "v3":
                        # epilogue fused across batch-tile pairs: one mult +
                        # one segmented reduce per 2 chunks of work
                        for tp in range(nt // 2):
                            wsb2 = wsp.tile([128, 2 * CH], MM_DT, tag="wsb2")
                            for tsub in range(2):
                                t = 2 * tp + tsub
                                ps = mp.tile([128, CH], F32)
                                for k in range(KH):
                                    nc.tensor.matmul(
                                        ps,
                                        hyps[k][:, t * 128 : (t + 1) * 128],
                                        w2c[:, k * CH : (k + 1) * CH],
                                        start=(k == 0),
                                        stop=(k == KH - 1),
                                    )
                                nc.scalar.activation(
                                    wsb2[:, tsub * CH : (tsub + 1) * CH], ps, copyf
                                )
                            prod2 = wsp.tile([128, 2 * CH], MM_DT, tag="prod2")
                            nc.vector.tensor_mul(
                                prod2,
                                wsb2,
                                inh_all[:, 2 * tp * CH : (2 * tp + 2) * CH],
                            )
                            nc.vector.tensor_reduce(
                                oacc3[:, 2 * tp : 2 * tp + 2, OPC * c : OPC * (c + 1)],
                                prod2.rearrange("p (t o i) -> p t o i", o=OPC, i=IN),
                                axis=mybir.AxisListType.X,
                                op=add,
                            )
                        continue
                    for t in range(nt):
                        ps = mp.tile([128, CH], F32)
                        for k in range(KH):
                            nc.tensor.matmul(
                                ps,
                                hyps[k][:, t * 128 : (t + 1) * 128],
                                w2c[:, k * CH : (k + 1) * CH],
                                start=(k == 0),
                                stop=(k == KH - 1),
                            )
                        if variant == "v7":
                            # fused (in0*1+0)*in1 multiply + per-partition sum
                            # on the custom-DVE path, straight from PSUM —
                            # no ScalarE copy, one DVE op per out-col
                            for orel in range(OPC):
                                o = OPC * c + orel
                                nc.vector.affine_mul_reduce(
                                    scratch[:, orel * IN : (orel + 1) * IN],
                                    oacc[:, t * OS + o : t * OS + o + 1],
                                    ps[:, orel * IN : (orel + 1) * IN],
                                    inh_all[:, t * CH + orel * IN : t * CH + (orel + 1) * IN],
                                    scale=1.0,
                                    bias=0.0,
                                )
                            continue
                        wsb = wsp.tile([128, CH], MM_DT)
                        nc.scalar.activation(wsb, ps, copyf)
                        if variant == "pe_act":
                            continue
                        use_ttr = variant == "v2ttr" or (
                            variant == "v2both" and t % 2 == 0
                        )
                        if use_ttr:
                            for orel in range(OPC):
                                o = OPC * c + orel
                                nc.vector.tensor_tensor_reduce(
                                    scratch[:, orel * IN : (orel + 1) * IN],
                                    wsb[:, orel * IN : (orel + 1) * IN],
                                    inh_all[:, t * CH + orel * IN : t * CH + (orel + 1) * IN],
                                    scale=1.0,
                                    scalar=0.0,
                                    op0=mult,
                                    op1=add,
                                    accum_out=oacc[:, t * OS + o : t * OS + o + 1],
                                )
                        else:
                            prod = wsp.tile([128, CH], MM_DT, tag="prod")
                            nc.vector.tensor_mul(
                                prod, wsb, inh_all[:, t * CH : (t + 1) * CH]
                            )
                            nc.vector.tensor_reduce(
                                oacc[:, t * OS + OPC * c : t * OS + OPC * (c + 1)],
                                prod.rearrange("p (o i) -> p o i", i=IN),
                                axis=mybir.AxisListType.X,
                                op=add,
                            )
            # final: add the (input_h @ b2 + bias) term
            nc.vector.tensor_add(oacc, oacc, cterm)

            for t in range(nt):
                nc.sync.dma_start(
                    out_d[t * 128 : (t + 1) * 128, :], oacc[:, t * OS : (t + 1) * OS]
                )

    nc.compile()
    return nc


def make_in_maps(inputs, b=B):
    """Host-side input prep: layout shuffles + dtype casts, per-core shards."""
    inh = np.asarray(inputs["input_h"], dtype=np.float32)
    z = np.asarray(inputs["condition_z"], dtype=np.float32)
    W1 = np.asarray(inputs["W1"], dtype=np.float32)
    b1 = np.asarray(inputs["b1"], dtype=np.float32)
    W2 = np.asarray(inputs["W2"], dtype=np.float32)
    b2 = np.asarray(inputs["b2"], dtype=np.float32)
    bias = np.asarray(inputs["bias"], dtype=np.float32).reshape(1, OUT)

    bf = NP_MM_DT
    zt = np.ascontiguousarray(z.T).astype(bf)  # [COND, b]
    w1h = W1.astype(bf)  # [COND, HID]
    b1t = np.ascontiguousarray(b1.reshape(KH, 128).T).astype(np.float32)  # [128, KH]
    inhb = np.concatenate([inh, inh], axis=1).astype(bf)  # [b, 2*IN]
    # pad the augmented rows to a full 128-partition k-tile (row 256 = ones,
    # rows 257..383 = zeros) so every SBUF tile/matmul uses 128 partitions
    ita = np.concatenate(
        [inh.T, np.ones((1, b), np.float32), np.zeros((127, b), np.float32)], axis=0
    ).astype(bf)
    W2r = W2.reshape(HID, IN, OUT)
    b2r = b2.reshape(IN, OUT)

    in_maps = []
    for c in range(NCORES):
        osl = slice(c * OS, (c + 1) * OS)
        # o-major columns: col = o_rel * IN + i
        w2s = (
            np.ascontiguousarray(W2r[:, :, osl].transpose(0, 2, 1))
            .reshape(HID, OS * IN)
            .astype(bf)
        )
        b2a = np.concatenate(
            [b2r[:, osl], bias[:, osl], np.zeros((127, OS), np.float32)], axis=0
        ).astype(bf)
        in_maps.append(
            {
                "zt": zt,
                "w1": w1h,
                "b1t": b1t,
                "inhb": inhb,
                "inhta": ita,
                "b2a": b2a,
                "w2s": w2s,
            }
        )
    return in_maps


_NC_CACHE = {}


def get_nc(b=B):
    if b not in _NC_CACHE:
        _NC_CACHE[b] = build(b)
    return _NC_CACHE[b]


def kernel(**inputs) -> np.ndarray:
    nc = get_nc(B)
    in_maps = make_in_maps(inputs, B)
    res = run_bass_kernel_spmd(nc, in_maps, core_ids=list(range(NCORES)))
    out = np.concatenate(
        [res.results[c]["out"] for c in range(NCORES)], axis=1
    )
    return np.ascontiguousarray(out.astype(np.float32))


# revision 33
# speedup vs baseline: 1.3482x; 1.0447x over previous
"""Trainium2 Bass kernel for APGLinear (hypernet-generated per-sample Linear).

Reference computation (B=4096, IN=256, OUT=256, COND=128, HID=512):
    hyp      = relu(condition_z @ W1 + b1)            # [B, HID]
    weight_S = (hyp @ W2 + b2).reshape(B, IN, OUT)    # 1 GB intermediate
    out      = einsum("bi,bio->bo", input_h, weight_S) + bias

Strategy:
  * Shard OUT across the 8 cores (32 columns each). Each core needs only a
    16 MB slice of W2, all of z / input_h, and computes the full hypernet
    activations locally (cheap). No collectives; host concatenates outputs.
  * The big matmul hyp @ W2_slice runs on the TensorEngine in bf16 with
    hyp^T tiles stationary; per-sample weights exist only in PSUM chunks of
    [128 samples, 2 out-cols x 256 in] (o-major layout).
  * The per-sample contraction sum_i input_h[b,i] * wS[b,i,o] is ONE fused
    custom-DVE op per output column (affine_mul_reduce: multiply + row-sum),
    reading wS straight from PSUM in f32 — no ScalarE downcast pass needed.
    (The ISA-level tensor_tensor_reduce would do the same but faults on this
    HW path; the v2mr fallback uses ScalarE-copy + tensor_mul +
    tensor_reduce instead.) The (input_h @ b2 + bias) term comes from a tiny
    augmented matmul and is added at the end.
"""

import os
import sys

import numpy as np

for _p in ("/opt/trn_rl_repo",):
    if os.path.isdir(_p) and _p not in sys.path:
        sys.path.append(_p)

import ml_dtypes  # noqa: E402

import concourse.bass as bass  # noqa: E402,F401
import concourse.tile as tile  # noqa: E402
from concourse import bacc, mybir  # noqa: E402
from concourse.bass_utils import run_bass_kernel_spmd  # noqa: E402

B = 4096
COND = 128
IN = 256
OUT = 256
HID = 512
NCORES = 8
OS = OUT // NCORES  # 32 out-cols per core
KH = HID // 128  # 4 contraction tiles for the big matmul
CH = 512  # psum chunk columns (= one PSUM bank of fp32)
OPC = CH // IN  # out-cols per chunk = 2

MM_DT = mybir.dt.bfloat16
NP_MM_DT = ml_dtypes.bfloat16
F32 = mybir.dt.float32


def build(b=B, variant="v7"):
    """Build + compile the SPMD single-core program (same on all 8 cores).

    variant: "v7" (default, HW-validated: fused affine_mul_reduce from PSUM,
    deep bufs) | "v2mr" (fallback: ScalarE copy + mul + segmented reduce) |
    "v8" (v2mr + deep bufs) | "v3"/"v6" (coarsened pipelines — slower) |
    "v2ttr"/"v2both" (ISA tensor_tensor_reduce — crashes real HW).
    """
    nt = b // 128  # batch tiles
    hbc = 512  # hyp-phase batch chunk (moving-operand cols)
    nbc = max(1, b // hbc)
    nch = (IN * OS) // CH  # 16 chunks per batch tile

    nc = bacc.Bacc("TRN2", target_bir_lowering=False, debug=False)

    zt_d = nc.dram_tensor("zt", [COND, b], MM_DT, kind="ExternalInput").ap()
    w1_d = nc.dram_tensor("w1", [COND, HID], MM_DT, kind="ExternalInput").ap()
    b1_d = nc.dram_tensor("b1t", [128, KH], F32, kind="ExternalInput").ap()
    # input_h duplicated along cols ([inh, inh]) in bf16, for the fused
    # multiply-reduce against o-major wS chunks
    inh_d = nc.dram_tensor("inhb", [b, CH], MM_DT, kind="ExternalInput").ap()
    ita_d = nc.dram_tensor("inhta", [IN + 128, b], MM_DT, kind="ExternalInput").ap()
    b2a_d = nc.dram_tensor("b2a", [IN + 128, OS], MM_DT, kind="ExternalInput").ap()
    w2_d = nc.dram_tensor("w2s", [HID, IN * OS], MM_DT, kind="ExternalInput").ap()
    out_d = nc.dram_tensor("out", [b, OS], F32, kind="ExternalOutput").ap()

    relu = mybir.ActivationFunctionType.Relu
    mult = mybir.AluOpType.mult
    add = mybir.AluOpType.add

    with tile.TileContext(nc) as tc:
        with tc.tile_pool(name="persist", bufs=1) as pp:
            zt = pp.tile([COND, b], MM_DT)
            nc.sync.dma_start(zt, zt_d)
            w1 = pp.tile([COND, HID], MM_DT)
            nc.sync.dma_start(w1, w1_d)
            b1t = pp.tile([128, KH], F32)
            nc.sync.dma_start(b1t, b1_d)
            ita0 = pp.tile([128, b], MM_DT)
            nc.sync.dma_start(ita0, ita_d[0:128, :])
            ita1 = pp.tile([128, b], MM_DT)
            nc.sync.dma_start(ita1, ita_d[128:256, :])
            ita2 = pp.tile([128, b], MM_DT)
            nc.sync.dma_start(ita2, ita_d[256:384, :])
            b2a0 = pp.tile([128, OS], MM_DT)
            nc.sync.dma_start(b2a0, b2a_d[0:128, :])
            b2a1 = pp.tile([128, OS], MM_DT)
            nc.sync.dma_start(b2a1, b2a_d[128:256, :])
            b2a2 = pp.tile([128, OS], MM_DT)
            nc.sync.dma_start(b2a2, b2a_d[256:384, :])

            inh_all = pp.tile([128, nt * CH], MM_DT)
            for t in range(nt):
                nc.sync.dma_start(
                    inh_all[:, t * CH : (t + 1) * CH], inh_d[t * 128 : (t + 1) * 128, :]
                )

            hyps = [pp.tile([128, b], MM_DT, name=f"hypt{k}") for k in range(KH)]
            oacc = pp.tile([128, nt * OS], F32)
            cterm = pp.tile([128, nt * OS], F32)
            if variant == "pe_act":
                nc.vector.memset(oacc, 0.0)

            # ---- hypernet layer 1: hyp^T = relu(W1^T @ z^T + b1) ----
            with tc.tile_pool(name="hpsum", bufs=4, space="PSUM") as hpp:
                for k in range(KH):
                    for c2 in range(nbc):
                        hp = hpp.tile([128, min(hbc, b)], F32, tag="hp")
                        bs = slice(c2 * hbc, c2 * hbc + min(hbc, b))
                        nc.tensor.matmul(
                            hp,
                            w1[:, k * 128 : (k + 1) * 128],
                            zt[:, bs],
                            start=True,
                            stop=True,
                        )
                        nc.scalar.activation(
                            hyps[k][:, bs], hp, relu, bias=b1t[:, k : k + 1], scale=1.0
                        )
                # ---- c-term: cterm = input_h @ b2_slice + bias_slice ----
                for t in range(nt):
                    cp = hpp.tile([128, OS], F32, tag="cp")
                    bsl = slice(t * 128, (t + 1) * 128)
                    nc.tensor.matmul(cp, ita0[:, bsl], b2a0, start=True, stop=False)
                    nc.tensor.matmul(cp, ita1[:, bsl], b2a1, start=False, stop=False)
                    nc.tensor.matmul(cp, ita2[:, bsl], b2a2, start=False, stop=True)
                    nc.vector.tensor_copy(cterm[:, t * OS : (t + 1) * OS], cp)

            # ---- main: wS chunks in PSUM -> bf16 SBUF (ScalarE) -> fused
            # multiply(+reduce) on the Vector engine ----
            scratch = pp.tile([128, CH], MM_DT)
            copyf = mybir.ActivationFunctionType.Copy
            deep = variant in ("v8", "v7", "v9")
            with (
                tc.tile_pool(name="w2p", bufs=4 if deep else 3) as w2p,
                tc.tile_pool(name="wsp", bufs=8 if deep else 4) as wsp,
                tc.tile_pool(name="mp", bufs=8 if deep else 6, space="PSUM") as mp,
            ):
                oacc3 = oacc.rearrange("p (t o) -> p t o", o=OS)
                if variant == "v6":
                    # k-grouped: one stationary load covers 4 chunk-matmuls
                    # (4 PSUM banks accumulate in parallel per k step)
                    gsz = 4
                    for g in range(nch // gsz):
                        w2g = w2p.tile([128, KH * gsz * CH], MM_DT, tag="w2g")
                        for k in range(KH):
                            nc.sync.dma_start(
                                w2g[:, k * gsz * CH : (k + 1) * gsz * CH],
                                w2_d[
                                    k * 128 : (k + 1) * 128,
                                    g * gsz * CH : (g + 1) * gsz * CH,
                                ],
                            )
                        for t in range(nt):
                            ps4 = mp.tile([128, gsz * CH], F32, tag="ps4", bufs=2)
                            for k in range(KH):
                                for cc in range(gsz):
                                    nc.tensor.matmul(
                                        ps4[:, cc * CH : (cc + 1) * CH],
                                        hyps[k][:, t * 128 : (t + 1) * 128],
                                        w2g[:, (k * gsz + cc) * CH : (k * gsz + cc + 1) * CH],
                                        start=(k == 0),
                                        stop=(k == KH - 1),
                                        skip_group_check=True,
                                    )
                            for cc in range(gsz):
                                c = g * gsz + cc
                                wsb6 = wsp.tile([128, CH], MM_DT, tag="wsb6")
                                nc.scalar.activation(
                                    wsb6, ps4[:, cc * CH : (cc + 1) * CH], copyf
                                )
                                prod6 = wsp.tile([128, CH], MM_DT, tag="prod6")
                                nc.vector.tensor_mul(
                                    prod6, wsb6, inh_all[:, t * CH : (t + 1) * CH]
                                )
                                nc.vector.tensor_reduce(
                                    oacc3[:, t, OPC * c : OPC * (c + 1)],
                                    prod6.rearrange("p (o i) -> p o i", i=IN),
                                    axis=mybir.AxisListType.X,
                                    op=add,
                                )
                    # the shared cterm add below the loop still runs
                    nch_eff = 0
                else:
                    nch_eff = nch
                for c in range(nch_eff):
                    w2c = w2p.tile([128, KH * CH], MM_DT)
                    for k in range(KH):
                        nc.sync.dma_start(
                            w2c[:, k * CH : (k + 1) * CH],
                            w2_d[k * 128 : (k + 1) * 128, c * CH : (c + 1) * CH],
                        )
                    if variant == "v3":
                        # epilogue fused across batch-tile pairs: one mult +
                        # one segmented reduce per 2 chunks of work
                        for tp in range(nt // 2):
                            wsb2 = wsp.tile([128, 2 * CH], MM_DT, tag="wsb2")
                            for tsub in range(2):
                                t = 2 * tp + tsub
                                ps = mp.tile([128, CH], F32)
                                for k in range(KH):
                                    nc.tensor.matmul(
                                        ps,
                                        hyps[k][:, t * 128 : (t + 1) * 128],
                                        w2c[:, k * CH : (k + 1) * CH],
                                        start=(k == 0),
                                        stop=(k == KH - 1),
                                    )
                                nc.scalar.activation(
                                    wsb2[:, tsub * CH : (tsub + 1) * CH], ps, copyf
                                )
                            prod2 = wsp.tile([128, 2 * CH], MM_DT, tag="prod2")
                            nc.vector.tensor_mul(
                                prod2,
                                wsb2,
                                inh_all[:, 2 * tp * CH : (2 * tp + 2) * CH],
                            )
                            nc.vector.tensor_reduce(
                                oacc3[:, 2 * tp : 2 * tp + 2, OPC * c : OPC * (c + 1)],
                                prod2.rearrange("p (t o i) -> p t o i", o=OPC, i=IN),
                                axis=mybir.AxisListType.X,
                                op=add,
                            )
                        continue
                    for t in range(nt):
                        ps = mp.tile([128, CH], F32)
                        for k in range(KH):
                            nc.tensor.matmul(
                                ps,
                                hyps[k][:, t * 128 : (t + 1) * 128],
                                w2c[:, k * CH : (k + 1) * CH],
                                start=(k == 0),
                                stop=(k == KH - 1),
                            )
                        if variant == "v9":
                            # like v7 but via a ScalarE bf16 downcast so the
                            # fused DVE op reads SBUF bf16 (2x-mode eligible)
                            wsb9 = wsp.tile([128, CH], MM_DT, tag="wsb9")
                            nc.scalar.activation(wsb9, ps, copyf)
                            for orel in range(OPC):
                                o = OPC * c + orel
                                nc.vector.affine_mul_reduce(
                                    scratch[:, orel * IN : (orel + 1) * IN],
                                    oacc[:, t * OS + o : t * OS + o + 1],
                                    wsb9[:, orel * IN : (orel + 1) * IN],
                                    inh_all[:, t * CH + orel * IN : t * CH + (orel + 1) * IN],
                                    scale=1.0,
                                    bias=0.0,
                                )
                            continue
                        if variant == "v7":
                            # fused (in0*1+0)*in1 multiply + per-partition sum
                            # on the custom-DVE path, straight from PSUM —
                            # no ScalarE copy, one DVE op per out-col
                            for orel in range(OPC):
                                o = OPC * c + orel
                                nc.vector.affine_mul_reduce(
                                    scratch[:, orel * IN : (orel + 1) * IN],
                                    oacc[:, t * OS + o : t * OS + o + 1],
                                    ps[:, orel * IN : (orel + 1) * IN],
                                    inh_all[:, t * CH + orel * IN : t * CH + (orel + 1) * IN],
                                    scale=1.0,
                                    bias=0.0,
                                )
                            continue
                        wsb = wsp.tile([128, CH], MM_DT)
                        nc.scalar.activation(wsb, ps, copyf)
                        if variant == "pe_act":
                            continue
                        use_ttr = variant == "v2ttr" or (
                            variant == "v2both" and t % 2 == 0
                        )
                        if use_ttr:
                            for orel in range(OPC):
                                o = OPC * c + orel
                                nc.vector.tensor_tensor_reduce(
                                    scratch[:, orel * IN : (orel + 1) * IN],
                                    wsb[:, orel * IN : (orel + 1) * IN],
                                    inh_all[:, t * CH + orel * IN : t * CH + (orel + 1) * IN],
                                    scale=1.0,
                                    scalar=0.0,
                                    op0=mult,
                                    op1=add,
                                    accum_out=oacc[:, t * OS + o : t * OS + o + 1],
                                )
                        else:
                            prod = wsp.tile([128, CH], MM_DT, tag="prod")
                            nc.vector.tensor_mul(
                                prod, wsb, inh_all[:, t * CH : (t + 1) * CH]
                            )
                            nc.vector.tensor_reduce(
                                oacc[:, t * OS + OPC * c : t * OS + OPC * (c + 1)],
                                prod.rearrange("p (o i) -> p o i", i=IN),
                                axis=mybir.AxisListType.X,
                                op=add,
                            )
            # final: add the (input_h @ b2 + bias) term
            nc.vector.tensor_add(oacc, oacc, cterm)

            for t in range(nt):
                nc.sync.dma_start(
                    out_d[t * 128 : (t + 1) * 128, :], oacc[:, t * OS : (t + 1) * OS]
                )

    nc.compile()
    return nc


def make_in_maps(inputs, b=B):
    """Host-side input prep: layout shuffles + dtype casts, per-core shards."""
    inh = np.asarray(inputs["input_h"], dtype=np.float32)
    z = np.asarray(inputs["condition_z"], dtype=np.float32)
    W1 = np.asarray(inputs["W1"], dtype=np.float32)
    b1 = np.asarray(inputs["b1"], dtype=np.float32)
    W2 = np.asarray(inputs["W2"], dtype=np.float32)
    b2 = np.asarray(inputs["b2"], dtype=np.float32)
    bias = np.asarray(inputs["bias"], dtype=np.float32).reshape(1, OUT)

    bf = NP_MM_DT
    zt = np.ascontiguousarray(z.T).astype(bf)  # [COND, b]
    w1h = W1.astype(bf)  # [COND, HID]
    b1t = np.ascontiguousarray(b1.reshape(KH, 128).T).astype(np.float32)  # [128, KH]
    inhb = np.concatenate([inh, inh], axis=1).astype(bf)  # [b, 2*IN]
    # pad the augmented rows to a full 128-partition k-tile (row 256 = ones,
    # rows 257..383 = zeros) so every SBUF tile/matmul uses 128 partitions
    ita = np.concatenate(
        [inh.T, np.ones((1, b), np.float32), np.zeros((127, b), np.float32)], axis=0
    ).astype(bf)
    W2r = W2.reshape(HID, IN, OUT)
    b2r = b2.reshape(IN, OUT)

    in_maps = []
    for c in range(NCORES):
        osl = slice(c * OS, (c + 1) * OS)
        # o-major columns: col = o_rel * IN + i
        w2s = (
            np.ascontiguousarray(W2r[:, :, osl].transpose(0, 2, 1))
            .reshape(HID, OS * IN)
            .astype(bf)
        )
        b2a = np.concatenate(
            [b2r[:, osl], bias[:, osl], np.zeros((127, OS), np.float32)], axis=0
        ).astype(bf)
        in_maps.append(
            {
                "zt": zt,
                "w1": w1h,
                "b1t": b1t,
                "inhb": inhb,
                "inhta": ita,
                "b2a": b2a,
                "w2s": w2s,
            }
        )
    return in_maps


_NC_CACHE = {}


def get_nc(b=B):
    if b not in _NC_CACHE:
        _NC_CACHE[b] = build(b)
    return _NC_CACHE[b]


def kernel(**inputs) -> np.ndarray:
    nc = get_nc(B)
    in_maps = make_in_maps(inputs, B)
    res = run_bass_kernel_spmd(nc, in_maps, core_ids=list(range(NCORES)))
    out = np.concatenate(
        [res.results[c]["out"] for c in range(NCORES)], axis=1
    )
    return np.ascontiguousarray(out.astype(np.float32))
